# revision 16
# baseline (speedup 1.0000x reference)
"""Causal self-attention Trainium2 kernel (8 NeuronCores).

Sharding: tensor-parallel over heads x data-parallel over batch.
Core c handles batch b = c // 4 and head group g = c % 4 (4 heads of 16).
Each core computes q/k/v projections for its heads, causal attention, and a
partial output projection (its 256 columns of the 1024-wide contraction);
the host sums the 4 partials per batch.

Schedule (v2): single interleaved stream engineered around two facts from
the v1 trace: (a) the ScalarE exp drain paces attention (~1us per kc), and
(b) the PE HAM clock re-throttles to 1.2 GHz whenever the PE micro-idles,
doubling every matmul. Fixes:
  - heads processed in two pair-sweeps per q-block so PSUM fits: 2 PV
    accumulator banks + 2x2-bank score tiles (rotation) + 2 filler banks.
  - scores for a head pair are two K=64 matmuls at base partitions 0/64
    emitted back-to-back -> co-run in opposite PE row groups.
  - one [128, 2, 512] exp per kc (halves ACT instruction overhead).
  - causal narrowing: diag chunk j only computes score/exp/PV columns
    >= min(128j, 256); the mask shrinks to one [128,256] = [zeros|triu]
    multiply on the band.
  - QKV projection of block b+1 and output projection of block b-1 are
    emitted as PE filler inside block b's attention loop so the PE never
    idles (keeps HAM warm) while ACT drains exps.
  - all PSUM drains on VectorE explicitly; den rows on ScalarE; recip +
    normalization on VectorE; partition broadcast on GpSimd.
"""

import collections

import ml_dtypes
import numpy as np

import concourse.bass as bass
from concourse import bacc
import concourse.mybir as mybir
import concourse.tile as tile
from concourse.bass_utils import run_bass_kernel_spmd

B, T, D, H = 2, 2048, 1024, 16
HD = D // H          # 64
HPC = 4              # heads per core
NCORES = 8
EQK = 2 * HPC * HD   # 512 rows of q+k per core
EV = HPC * HD        # 256 rows of v per core
TB = 512             # t/q block
NTB = T // TB        # 4
TC = 128             # t chunk
NTC = T // TC        # 16
DCH = D // 128       # 8 contraction chunks
F32 = mybir.dt.float32
BF16 = mybir.dt.bfloat16
EXP = mybir.ActivationFunctionType.Exp

_cache = {}


def _ensure_ntff_hook():
    """The agent image's antenv lacks axon_hooks; fabricate it so
    run_bass_kernel_spmd(trace=True) can capture NTFF profiles."""
    import sys
    import types
    try:
        import antenv.axon_hooks  # noqa: F401
        return
    except ImportError:
        pass
    try:
        import antenv
        from trn_agent_boot.trn_boot import _ntff_profile_via_ctypes
        hook = {"h": _ntff_profile_via_ctypes("/opt/axon/libaxon_pjrt.so")}
        m = types.ModuleType("antenv.axon_hooks")
        m.get_axon_ntff_profile_hook = lambda: hook["h"]
        m.set_axon_ntff_profile_hook = lambda h: hook.update(h=h)
        sys.modules["antenv.axon_hooks"] = m
        antenv.axon_hooks = m
    except Exception:
        pass


def _build_nc():
    nc = bacc.Bacc("TRN2", target_bir_lowering=False, debug=False,
                  num_devices=NCORES)
    xT = nc.dram_tensor("xT", [D, T], BF16, kind="ExternalInput")
    wqk = nc.dram_tensor("wqk", [D, EQK], BF16, kind="ExternalInput")
    wv = nc.dram_tensor("wv", [D, EV], BF16, kind="ExternalInput")
    wp = nc.dram_tensor("wp", [EV, D], BF16, kind="ExternalInput")
    # [zeros(128x128) | triu(128x128)] causal band mask
    masks = nc.dram_tensor("masks", [128, 256], BF16, kind="ExternalInput")
    onesd = nc.dram_tensor("onesd", [128, HD], BF16, kind="ExternalInput")
    out = nc.dram_tensor("out", [T, D], BF16, kind="ExternalOutput")

    with tile.TileContext(nc) as tc:
        with (
            nc.allow_low_precision(reason="fp32r matmul inputs; psum stays fp32"),
            tc.tile_pool(name="persist", bufs=1) as persist,
            tc.tile_pool(name="xin", bufs=2) as xin,
            tc.tile_pool(name="work", bufs=4) as work,
            tc.tile_pool(name="probsp", bufs=4) as probsp,
            tc.tile_pool(name="outp", bufs=3) as outp,
            tc.tile_pool(name="ps_s", bufs=2, space="PSUM") as ps_s,
            tc.tile_pool(name="ps_pv", bufs=2, space="PSUM") as ps_pv_pool,
            tc.tile_pool(name="ps_mm", bufs=2, space="PSUM") as ps_mm,
        ):
            # ---- persistent SBUF tensors / input DMAs ----
            # Big DMAs are split per 128-row chunk so the first qk chain can
            # start after ~256KB arrives instead of waiting for 4MB.
            wqk_sb = persist.tile([128, DCH, EQK], BF16)   # 16KB/part
            x_tiles = {}

            def dma_x(b, split=False):
                t = xin.tile([128, DCH, TB], BF16, tag="x")
                if split:
                    for dc in range(DCH):
                        nc.sync.dma_start(
                            t[:, dc, :],
                            xT[128 * dc:128 * (dc + 1),
                               b * TB:(b + 1) * TB])
                else:
                    nc.sync.dma_start(
                        t[:], xT[:, b * TB:(b + 1) * TB]
                        .rearrange("(c p) t -> p c t", p=128))
                x_tiles[b] = t

            x_tiles[0] = xin.tile([128, DCH, TB], BF16, tag="x", name="x0")
            for dc in range(DCH):
                nc.sync.dma_start(
                    wqk_sb[:, dc, :], wqk[128 * dc:128 * (dc + 1), :])
                nc.sync.dma_start(
                    x_tiles[0][:, dc, :], xT[128 * dc:128 * (dc + 1), 0:TB])
            wv_sb = persist.tile([128, DCH, EV], BF16)     # 8KB/part
            for dc in range(DCH):
                nc.sync.dma_start(
                    wv_sb[:, dc, :], wv[128 * dc:128 * (dc + 1), :])
            mask_sb = persist.tile([128, 256], BF16)
            nc.sync.dma_start(mask_sb[:], masks[:, :])
            wp_sb = persist.tile([128, 2, D], BF16)        # 8KB/part
            nc.sync.dma_start(wp_sb[:], wp.rearrange("(c p) e -> p c e", p=128))
            dma_x(1)

            # warm the ACT exp table set while QKV(0) runs
            warm = work.tile([1, 8], F32, tag="warm")
            nc.scalar.activation(warm[:], mask_sb[0:1, 0:8], EXP)

            # warm the PE HAM clock gate while the first DMAs trickle in:
            # dummy matmuls against the first-arriving wqk chunk keep the PE
            # busy through the 3.4us activity window so real matmuls run at
            # 2.4 GHz from the start.
            for i in range(56):
                ps_d = ps_s.tile([128, 2, TB], F32, tag="s", name="ps_d")
                nc.tensor.matmul(ps_d[:, 0, 0:256], wqk_sb[:, 0, 0:128],
                                 wqk_sb[:, 0, 0:256], start=True, stop=True)

            # qkT[e, t]: 4 chunks of 128 e-rows (q heads 01, q heads 23,
            # k heads 01, k heads 23), each [128, T]
            qkT = [persist.tile([128, T], BF16, tag=f"qkT{i}", name=f"qkT{i}")
                   for i in range(4)]
            # v_sb[t_chunk]: [128, h, 65]; col 64 of each head slot is 1.0
            v_sb = [persist.tile([128, HPC, HD + 1], BF16, tag=f"v{i}",
                                name=f"v{i}")
                    for i in range(NTC)]
            # yT: unnormalized-then-normalized attention output, [hd_all, t]
            yT = [persist.tile([128, T], BF16, tag=f"yT{i}", name=f"yT{i}")
                  for i in range(2)]

            def qT_ap(h):  # [64, T]
                return qkT[h // 2][64 * (h % 2):64 * (h % 2) + 64, :]

            def kT_ap(h):  # [64, T]
                return qkT[2 + h // 2][64 * (h % 2):64 * (h % 2) + 64, :]

            # ---------------- chain emitters (filler units) ----------------
            def emit_qk_chain(b, ec):
                ps = ps_mm.tile([128, TB], F32, tag="mm", name="ps_qk")
                for dc in range(DCH):
                    nc.tensor.matmul(
                        ps[:],
                        (wqk_sb[:, dc, 128 * ec:128 * (ec + 1)]),
                        (x_tiles[b][:, dc, :]),
                        start=(dc == 0), stop=(dc == DCH - 1))
                nc.vector.tensor_copy(qkT[ec][:, b * TB:(b + 1) * TB], ps[:])

            def emit_v_chain(b, t2):
                tc_i = 4 * b + t2
                ps = ps_mm.tile([128, TB], F32, tag="mm", name="ps_v")
                for dc in range(DCH):
                    nc.tensor.matmul(
                        ps[:, 0:EV],
                        (x_tiles[b][:, dc, 128 * t2:128 * (t2 + 1)]),
                        (wv_sb[:, dc, :]),
                        start=(dc == 0), stop=(dc == DCH - 1))
                nc.vector.tensor_copy(
                    v_sb[tc_i][:, :, 0:HD],
                    ps[:, 0:EV].rearrange("p (h f) -> p h f", h=HPC))
                nc.sync.dma_start(v_sb[tc_i][:, :, HD], onesd[:, 0:HPC])

            def emit_proj_chain(tc_i, e, drain_on_act=False):
                ps = ps_mm.tile([128, TB], F32, tag="mm", name="ps_proj")
                for c in range(2):
                    nc.tensor.matmul(
                        ps[:],
                        (yT[c][:, 128 * tc_i:128 * (tc_i + 1)]),
                        (wp_sb[:, c, 512 * e:512 * (e + 1)]),
                        start=(c == 0), stop=(c == 1))
                o_sb = outp.tile([128, TB], BF16, tag="o")
                if drain_on_act:
                    # ACT is idle after the last exp; keep DVE free for the
                    # tail-critical normalization chain.
                    nc.scalar.copy(o_sb[:], ps[:])
                else:
                    nc.vector.tensor_copy(o_sb[:], ps[:])
                nc.sync.dma_start(
                    out[128 * tc_i:128 * (tc_i + 1),
                        512 * e:512 * (e + 1)], o_sb[:])

            # qkv_fill entries are (block, thunk) and must run before that
            # block's attention; proj_fill can run whenever.
            qkv_fill = collections.deque()
            proj_fill = collections.deque()

            def pop_filler():
                if qkv_fill:
                    qkv_fill.popleft()[1]()
                elif proj_fill:
                    proj_fill.popleft()()

            # ---------------- prologue: QKV(0) ----------------
            for ec in range(4):
                emit_qk_chain(0, ec)
            for t2 in range(4):
                emit_v_chain(0, t2)

            # ---------------- main loop over q-blocks ----------------
            for b in range(NTB):
                nk = 4 * b + 4
                if b + 2 < NTB:
                    dma_x(b + 2)
                if b + 1 < NTB:
                    for ec in range(4):
                        qkv_fill.append(
                            (b + 1,
                             (lambda bb=b + 1, e=ec: emit_qk_chain(bb, e))))
                    for t2 in range(4):
                        qkv_fill.append(
                            (b + 1,
                             (lambda bb=b + 1, t=t2: emit_v_chain(bb, t))))
                if b >= 1:
                    for tci in range(4 * (b - 1), 4 * b):
                        for e in range(2):
                            proj_fill.append(
                                lambda t=tci, ee=e: emit_proj_chain(t, ee))
                # anything queued for block <= b must be emitted before the
                # sweeps that consume its outputs
                while qkv_fill and qkv_fill[0][0] <= b:
                    qkv_fill.popleft()[1]()

                for pair in range(2):
                    pvs = [ps_pv_pool.tile([HD + 1, TB], F32, tag="pv",
                                           name=f"pv{h2}")
                           for h2 in range(2)]
                    for kc in range(nk):
                        j = kc - 4 * b
                        c0 = 0 if j < 0 else min(128 * j, 256)
                        sc = ps_s.tile([128, 2, TB], F32, tag="s", name="sc")
                        for h2 in range(2):
                            h = 2 * pair + h2
                            nc.tensor.matmul(
                                sc[:, h2, c0:TB],
                                (kT_ap(h)[:, 128 * kc:128 * (kc + 1)]),
                                (qT_ap(h)[:, b * TB + c0:(b + 1) * TB]),
                                start=True, stop=True)
                        pr = probsp.tile([128, 2, TB], BF16, tag="p",
                                         name="probs")
                        nc.scalar.activation(
                            pr[:, :, c0:TB], sc[:, :, c0:TB], EXP,
                            scale=1.0 / np.sqrt(HD))
                        if j >= 0:
                            # band mask: j<3 -> triu on [c0,c0+128);
                            # j==3 -> [zeros|triu] on [256,512)
                            w = 256 if j == 3 else 128
                            ms = 0 if j == 3 else 128
                            for h2 in range(2):
                                nc.vector.tensor_mul(
                                    pr[:, h2, c0:c0 + w],
                                    pr[:, h2, c0:c0 + w],
                                    mask_sb[:, ms:ms + w])
                        for h2 in range(2):
                            nc.tensor.matmul(
                                pvs[h2][:, c0:TB],
                                (v_sb[kc][:, 2 * pair + h2, :]),
                                (pr[:, h2, c0:TB]),
                                start=(kc == 0), stop=(kc == nk - 1))
                        pace = 2 if b == NTB - 1 else 3
                        if kc % pace == pace - 1:
                            pop_filler()
                    # sweep drain: unnormalized yT copy first (frees PSUM),
                    # then normalize in SBUF off the bank critical path.
                    # High priority so the engines run these ahead of queued
                    # filler drains (the b=3 norm gates the last projections).
                    with tc.high_priority():
                        for h2 in range(2):
                            h = 2 * pair + h2
                            yslice = yT[h // 2][
                                64 * (h % 2):64 * (h % 2) + 64,
                                b * TB:(b + 1) * TB]
                            nc.vector.tensor_copy(yslice, pvs[h2][0:HD, :])
                            den = work.tile([1, TB], F32, tag="den",
                                            name=f"den{h2}")
                            nc.scalar.copy(den[:], pvs[h2][HD:HD + 1, :])
                            rec = work.tile([1, TB], F32, tag="rec",
                                            name=f"rec{h2}")
                            nc.vector.reciprocal_approx_fast(rec[:], den[:])
                            rec16 = work.tile([1, TB], BF16, tag="rec16",
                                              name=f"rec16_{h2}")
                            nc.vector.tensor_copy(rec16[:], rec[:])
                            bc = work.tile([128, TB], BF16, tag="bc")
                            nc.gpsimd.partition_broadcast(bc[:], rec16[:])
                            off = 64 * (h % 2)
                            nc.vector.tensor_mul(yslice, yslice,
                                                 bc[off:off + 64, :])

            # ---------------- epilogue ----------------
            # dummy matmuls the scheduler can drop into the PE-idle window
            # while the last exps/norms drain, keeping the HAM clock warm
            # for the final projections
            for i in range(16):
                ps_d2 = ps_s.tile([128, 2, TB], F32, tag="s", name="ps_d2")
                nc.tensor.matmul(ps_d2[:, 0, 0:256], wqk_sb[:, 0, 0:128],
                                 wqk_sb[:, 0, 0:256], start=True, stop=True)
            for tci in range(4 * (NTB - 1), 4 * NTB):
                for e in range(2):
                    proj_fill.append(
                        lambda t=tci, ee=e: emit_proj_chain(t, ee, True))
            while qkv_fill or proj_fill:
                pop_filler()
    nc.compile()
    return nc


def _mask_np():
    m = np.zeros((128, 256), dtype=np.float32)
    m[:, 128:] = np.triu(np.ones((128, 128), dtype=np.float32))
    return m


def _prep_in_maps(x, w_qkv, w_proj):
    bf = ml_dtypes.bfloat16
    mask = _mask_np()
    in_maps = []
    for c in range(NCORES):
        b, g = c // 4, c % 4
        heads = slice(g * HPC * HD, (g + 1) * HPC * HD)      # 256 rows
        wq = w_qkv[0 * D:1 * D][heads]                        # [256, 1024]
        wk = w_qkv[1 * D:2 * D][heads]
        wv = w_qkv[2 * D:3 * D][heads]
        in_maps.append({
            "xT": np.ascontiguousarray(x[b].T).astype(bf),    # [1024, 2048]
            "wqk": np.ascontiguousarray(
                np.concatenate([wq, wk], axis=0).T).astype(bf),
            "wv": np.ascontiguousarray(wv.T).astype(bf),      # [1024, 256]
            "wp": np.ascontiguousarray(w_proj[:, heads].T).astype(bf),
            "masks": mask.astype(bf),
            "onesd": np.ones((128, HD), dtype=bf),
        })
    return in_maps


def kernel(x, w_qkv, w_proj, _trace=False):
    x = np.asarray(x, dtype=np.float32)
    w_qkv = np.asarray(w_qkv, dtype=np.float32)
    w_proj = np.asarray(w_proj, dtype=np.float32)
    if _trace:
        _ensure_ntff_hook()
    if "nc" not in _cache:
        _cache["nc"] = _build_nc()
    nc = _cache["nc"]
    in_maps = _prep_in_maps(x, w_qkv, w_proj)
    res = run_bass_kernel_spmd(nc, in_maps, list(range(NCORES)),
                               trace=_trace)
    out = np.zeros((B, T, D), dtype=np.float32)
    for c in range(NCORES):
        out[c // 4] += np.asarray(res.results[c]["out"], dtype=np.float32)
    if _trace:
        _cache["last_result"] = res
    return out


# revision 19
# speedup vs baseline: 1.0278x; 1.0278x over previous
"""Causal self-attention Trainium2 kernel (8 NeuronCores).

Sharding: tensor-parallel over heads x data-parallel over batch.
Core c handles batch b = c // 4 and head group g = c % 4 (4 heads of 16).
Each core computes q/k/v projections for its heads, causal attention, and a
partial output projection (its 256 columns of the 1024-wide contraction);
the host sums the 4 partials per batch.

Schedule (v2): single interleaved stream engineered around two facts from
the v1 trace: (a) the ScalarE exp drain paces attention (~1us per kc), and
(b) the PE HAM clock re-throttles to 1.2 GHz whenever the PE micro-idles,
doubling every matmul. Fixes:
  - heads processed in two pair-sweeps per q-block so PSUM fits: 2 PV
    accumulator banks + 2x2-bank score tiles (rotation) + 2 filler banks.
  - scores for a head pair are two K=64 matmuls at base partitions 0/64
    emitted back-to-back -> co-run in opposite PE row groups.
  - one [128, 2, 512] exp per kc (halves ACT instruction overhead).
  - causal narrowing: diag chunk j only computes score/exp/PV columns
    >= min(128j, 256); the mask shrinks to one [128,256] = [zeros|triu]
    multiply on the band.
  - QKV projection of block b+1 and output projection of block b-1 are
    emitted as PE filler inside block b's attention loop so the PE never
    idles (keeps HAM warm) while ACT drains exps.
  - all PSUM drains on VectorE explicitly; den rows on ScalarE; recip +
    normalization on VectorE; partition broadcast on GpSimd.
"""

import collections

import ml_dtypes
import numpy as np

import concourse.bass as bass
from concourse import bacc
import concourse.mybir as mybir
import concourse.tile as tile
from concourse.bass_utils import run_bass_kernel_spmd

B, T, D, H = 2, 2048, 1024, 16
HD = D // H          # 64
HPC = 4              # heads per core
NCORES = 8
EQK = 2 * HPC * HD   # 512 rows of q+k per core
EV = HPC * HD        # 256 rows of v per core
TB = 512             # t/q block
NTB = T // TB        # 4
TC = 128             # t chunk
NTC = T // TC        # 16
DCH = D // 128       # 8 contraction chunks
F32 = mybir.dt.float32
BF16 = mybir.dt.bfloat16
EXP = mybir.ActivationFunctionType.Exp

_cache = {}


def _ensure_ntff_hook():
    """The agent image's antenv lacks axon_hooks; fabricate it so
    run_bass_kernel_spmd(trace=True) can capture NTFF profiles."""
    import sys
    import types
    try:
        import antenv.axon_hooks  # noqa: F401
        return
    except ImportError:
        pass
    try:
        import antenv
        from trn_agent_boot.trn_boot import _ntff_profile_via_ctypes
        hook = {"h": _ntff_profile_via_ctypes("/opt/axon/libaxon_pjrt.so")}
        m = types.ModuleType("antenv.axon_hooks")
        m.get_axon_ntff_profile_hook = lambda: hook["h"]
        m.set_axon_ntff_profile_hook = lambda h: hook.update(h=h)
        sys.modules["antenv.axon_hooks"] = m
        antenv.axon_hooks = m
    except Exception:
        pass


def _build_nc():
    nc = bacc.Bacc("TRN2", target_bir_lowering=False, debug=False,
                  num_devices=NCORES)
    xT = nc.dram_tensor("xT", [D, T], BF16, kind="ExternalInput")
    wqk = nc.dram_tensor("wqk", [D, EQK], BF16, kind="ExternalInput")
    wv = nc.dram_tensor("wv", [D, EV], BF16, kind="ExternalInput")
    wp = nc.dram_tensor("wp", [EV, D], BF16, kind="ExternalInput")
    # [zeros(128x128) | triu(128x128)] causal band mask
    masks = nc.dram_tensor("masks", [128, 256], BF16, kind="ExternalInput")
    onesd = nc.dram_tensor("onesd", [128, HD], BF16, kind="ExternalInput")
    out = nc.dram_tensor("out", [T, D], BF16, kind="ExternalOutput")

    with tile.TileContext(nc) as tc:
        with (
            nc.allow_low_precision(reason="fp32r matmul inputs; psum stays fp32"),
            tc.tile_pool(name="persist", bufs=1) as persist,
            tc.tile_pool(name="xin", bufs=2) as xin,
            tc.tile_pool(name="work", bufs=4) as work,
            tc.tile_pool(name="probsp", bufs=4) as probsp,
            tc.tile_pool(name="outp", bufs=3) as outp,
            tc.tile_pool(name="ps_s", bufs=2, space="PSUM") as ps_s,
            tc.tile_pool(name="ps_pv", bufs=2, space="PSUM") as ps_pv_pool,
            tc.tile_pool(name="ps_mm", bufs=2, space="PSUM") as ps_mm,
        ):
            # ---- persistent SBUF tensors / input DMAs ----
            # Big DMAs are split per 128-row chunk so the first qk chain can
            # start after ~256KB arrives instead of waiting for 4MB.
            wqk_sb = persist.tile([128, DCH, EQK], BF16)   # 16KB/part
            x_tiles = {}

            def dma_x(b, split=False):
                t = xin.tile([128, DCH, TB], BF16, tag="x")
                if split:
                    for dc in range(DCH):
                        nc.sync.dma_start(
                            t[:, dc, :],
                            xT[128 * dc:128 * (dc + 1),
                               b * TB:(b + 1) * TB])
                else:
                    nc.sync.dma_start(
                        t[:], xT[:, b * TB:(b + 1) * TB]
                        .rearrange("(c p) t -> p c t", p=128))
                x_tiles[b] = t

            x_tiles[0] = xin.tile([128, DCH, TB], BF16, tag="x", name="x0")
            for dc in range(DCH):
                nc.sync.dma_start(
                    wqk_sb[:, dc, :], wqk[128 * dc:128 * (dc + 1), :])
                nc.sync.dma_start(
                    x_tiles[0][:, dc, :], xT[128 * dc:128 * (dc + 1), 0:TB])
            wv_sb = persist.tile([128, DCH, EV], BF16)     # 8KB/part
            for dc in range(DCH):
                nc.sync.dma_start(
                    wv_sb[:, dc, :], wv[128 * dc:128 * (dc + 1), :])
            mask_sb = persist.tile([128, 256], BF16)
            nc.sync.dma_start(mask_sb[:], masks[:, :])
            wp_sb = persist.tile([128, 2, D], BF16)        # 8KB/part
            nc.sync.dma_start(wp_sb[:], wp.rearrange("(c p) e -> p c e", p=128))
            dma_x(1)

            # warm the ACT exp table set while QKV(0) runs
            warm = work.tile([1, 8], F32, tag="warm")
            nc.scalar.activation(warm[:], mask_sb[0:1, 0:8], EXP)

            # qkT[e, t]: 4 chunks of 128 e-rows (q heads 01, q heads 23,
            # k heads 01, k heads 23), each [128, T]
            qkT = [persist.tile([128, T], BF16, tag=f"qkT{i}", name=f"qkT{i}")
                   for i in range(4)]
            # v_sb[t_chunk]: [128, h, 65]; col 64 of each head slot is 1.0
            v_sb = [persist.tile([128, HPC, HD + 1], BF16, tag=f"v{i}",
                                name=f"v{i}")
                    for i in range(NTC)]
            # yT: unnormalized-then-normalized attention output, [hd_all, t]
            yT = [persist.tile([128, T], BF16, tag=f"yT{i}", name=f"yT{i}")
                  for i in range(2)]

            def qT_ap(h):  # [64, T]
                return qkT[h // 2][64 * (h % 2):64 * (h % 2) + 64, :]

            def kT_ap(h):  # [64, T]
                return qkT[2 + h // 2][64 * (h % 2):64 * (h % 2) + 64, :]

            # ---------------- chain emitters (filler units) ----------------
            def emit_qk_chain(b, ec):
                ps = ps_mm.tile([128, TB], F32, tag="mm", name="ps_qk")
                for dc in range(DCH):
                    nc.tensor.matmul(
                        ps[:],
                        (wqk_sb[:, dc, 128 * ec:128 * (ec + 1)]),
                        (x_tiles[b][:, dc, :]),
                        start=(dc == 0), stop=(dc == DCH - 1))
                nc.vector.tensor_copy(qkT[ec][:, b * TB:(b + 1) * TB], ps[:])

            def emit_v_chain(b, t2):
                tc_i = 4 * b + t2
                ps = ps_mm.tile([128, TB], F32, tag="mm", name="ps_v")
                for dc in range(DCH):
                    nc.tensor.matmul(
                        ps[:, 0:EV],
                        (x_tiles[b][:, dc, 128 * t2:128 * (t2 + 1)]),
                        (wv_sb[:, dc, :]),
                        start=(dc == 0), stop=(dc == DCH - 1))
                nc.vector.tensor_copy(
                    v_sb[tc_i][:, :, 0:HD],
                    ps[:, 0:EV].rearrange("p (h f) -> p h f", h=HPC))
                nc.sync.dma_start(v_sb[tc_i][:, :, HD], onesd[:, 0:HPC])

            def emit_proj_chain(tc_i, e, drain_on_act=False):
                ps = ps_mm.tile([128, TB], F32, tag="mm", name="ps_proj")
                for c in range(2):
                    nc.tensor.matmul(
                        ps[:],
                        (yT[c][:, 128 * tc_i:128 * (tc_i + 1)]),
                        (wp_sb[:, c, 512 * e:512 * (e + 1)]),
                        start=(c == 0), stop=(c == 1))
                o_sb = outp.tile([128, TB], BF16, tag="o")
                if drain_on_act:
                    # ACT is idle after the last exp; keep DVE free for the
                    # tail-critical normalization chain.
                    nc.scalar.copy(o_sb[:], ps[:])
                else:
                    nc.vector.tensor_copy(o_sb[:], ps[:])
                nc.sync.dma_start(
                    out[128 * tc_i:128 * (tc_i + 1),
                        512 * e:512 * (e + 1)], o_sb[:])

            # qkv_fill entries are (block, thunk) and must run before that
            # block's attention; proj_fill can run whenever.
            qkv_fill = collections.deque()
            proj_fill = collections.deque()

            def pop_filler():
                if qkv_fill:
                    qkv_fill.popleft()[1]()
                elif proj_fill:
                    proj_fill.popleft()()

            # warm the PE HAM clock gate while the first DMAs trickle in:
            # a contiguous dummy block BEFORE any real PSUM traffic (safe),
            # sized under the natural DMA wait so it barely delays chain 0.
            for i in range(24):
                ps_d = ps_s.tile([128, 2, TB], F32, tag="s", name="ps_d")
                nc.tensor.matmul(ps_d[:, 0, 0:256], wqk_sb[:, 0, 0:128],
                                 wqk_sb[:, 0, 0:256], start=True, stop=True)

            # ---------------- prologue: QKV(0) ----------------
            for ec in range(4):
                emit_qk_chain(0, ec)
            for t2 in range(4):
                emit_v_chain(0, t2)

            # ---------------- main loop over q-blocks ----------------
            for b in range(NTB):
                nk = 4 * b + 4
                if b + 2 < NTB:
                    dma_x(b + 2)
                if b + 1 < NTB:
                    for ec in range(4):
                        qkv_fill.append(
                            (b + 1,
                             (lambda bb=b + 1, e=ec: emit_qk_chain(bb, e))))
                    for t2 in range(4):
                        qkv_fill.append(
                            (b + 1,
                             (lambda bb=b + 1, t=t2: emit_v_chain(bb, t))))
                if b >= 1:
                    for tci in range(4 * (b - 1), 4 * b):
                        for e in range(2):
                            proj_fill.append(
                                lambda t=tci, ee=e: emit_proj_chain(t, ee))
                # anything queued for block <= b must be emitted before the
                # sweeps that consume its outputs
                while qkv_fill and qkv_fill[0][0] <= b:
                    qkv_fill.popleft()[1]()

                for pair in range(2):
                    pvs = [ps_pv_pool.tile([HD + 1, TB], F32, tag="pv",
                                           name=f"pv{h2}")
                           for h2 in range(2)]
                    for kc in range(nk):
                        j = kc - 4 * b
                        c0 = 0 if j < 0 else min(128 * j, 256)
                        sc = ps_s.tile([128, 2, TB], F32, tag="s", name="sc")
                        for h2 in range(2):
                            h = 2 * pair + h2
                            nc.tensor.matmul(
                                sc[:, h2, c0:TB],
                                (kT_ap(h)[:, 128 * kc:128 * (kc + 1)]),
                                (qT_ap(h)[:, b * TB + c0:(b + 1) * TB]),
                                start=True, stop=True)
                        pr = probsp.tile([128, 2, TB], BF16, tag="p",
                                         name="probs")
                        nc.scalar.activation(
                            pr[:, :, c0:TB], sc[:, :, c0:TB], EXP,
                            scale=1.0 / np.sqrt(HD))
                        if j >= 0:
                            # band mask: j<3 -> triu on [c0,c0+128);
                            # j==3 -> [zeros|triu] on [256,512)
                            w = 256 if j == 3 else 128
                            ms = 0 if j == 3 else 128
                            for h2 in range(2):
                                nc.vector.tensor_mul(
                                    pr[:, h2, c0:c0 + w],
                                    pr[:, h2, c0:c0 + w],
                                    mask_sb[:, ms:ms + w])
                        for h2 in range(2):
                            nc.tensor.matmul(
                                pvs[h2][:, c0:TB],
                                (v_sb[kc][:, 2 * pair + h2, :]),
                                (pr[:, h2, c0:TB]),
                                start=(kc == 0), stop=(kc == nk - 1))
                        pace = 2 if b == NTB - 1 else 3
                        if kc % pace == pace - 1:
                            pop_filler()
                    # sweep drain: unnormalized yT copy first (frees PSUM),
                    # then normalize in SBUF off the bank critical path.
                    # High priority so the engines run these ahead of queued
                    # filler drains (the b=3 norm gates the last projections).
                    with tc.high_priority():
                        for h2 in range(2):
                            h = 2 * pair + h2
                            yslice = yT[h // 2][
                                64 * (h % 2):64 * (h % 2) + 64,
                                b * TB:(b + 1) * TB]
                            nc.vector.tensor_copy(yslice, pvs[h2][0:HD, :])
                            den = work.tile([1, TB], F32, tag="den",
                                            name=f"den{h2}")
                            nc.scalar.copy(den[:], pvs[h2][HD:HD + 1, :])
                            rec = work.tile([1, TB], F32, tag="rec",
                                            name=f"rec{h2}")
                            nc.vector.reciprocal_approx_fast(rec[:], den[:])
                            rec16 = work.tile([1, TB], BF16, tag="rec16",
                                              name=f"rec16_{h2}")
                            nc.vector.tensor_copy(rec16[:], rec[:])
                            bc = work.tile([128, TB], BF16, tag="bc")
                            nc.gpsimd.partition_broadcast(bc[:], rec16[:])
                            off = 64 * (h % 2)
                            nc.vector.tensor_mul(yslice, yslice,
                                                 bc[off:off + 64, :])

            # ---------------- epilogue ----------------
            for tci in range(4 * (NTB - 1), 4 * NTB):
                for e in range(2):
                    proj_fill.append(
                        lambda t=tci, ee=e: emit_proj_chain(t, ee, True))
            while qkv_fill or proj_fill:
                pop_filler()

    nc.compile()
    return nc


def _mask_np():
    m = np.zeros((128, 256), dtype=np.float32)
    m[:, 128:] = np.triu(np.ones((128, 128), dtype=np.float32))
    return m


def _prep_in_maps(x, w_qkv, w_proj):
    bf = ml_dtypes.bfloat16
    mask = _mask_np()
    in_maps = []
    for c in range(NCORES):
        b, g = c // 4, c % 4
        heads = slice(g * HPC * HD, (g + 1) * HPC * HD)      # 256 rows
        wq = w_qkv[0 * D:1 * D][heads]                        # [256, 1024]
        wk = w_qkv[1 * D:2 * D][heads]
        wv = w_qkv[2 * D:3 * D][heads]
        in_maps.append({
            "xT": np.ascontiguousarray(x[b].T).astype(bf),    # [1024, 2048]
            "wqk": np.ascontiguousarray(
                np.concatenate([wq, wk], axis=0).T).astype(bf),
            "wv": np.ascontiguousarray(wv.T).astype(bf),      # [1024, 256]
            "wp": np.ascontiguousarray(w_proj[:, heads].T).astype(bf),
            "masks": mask.astype(bf),
            "onesd": np.ones((128, HD), dtype=bf),
        })
    return in_maps


def kernel(x, w_qkv, w_proj, _trace=False):
    x = np.asarray(x, dtype=np.float32)
    w_qkv = np.asarray(w_qkv, dtype=np.float32)
    w_proj = np.asarray(w_proj, dtype=np.float32)
    if _trace:
        _ensure_ntff_hook()
    if "nc" not in _cache:
        _cache["nc"] = _build_nc()
    nc = _cache["nc"]
    in_maps = _prep_in_maps(x, w_qkv, w_proj)
    res = run_bass_kernel_spmd(nc, in_maps, list(range(NCORES)),
                               trace=_trace)
    out = np.zeros((B, T, D), dtype=np.float32)
    for c in range(NCORES):
        out[c // 4] += np.asarray(res.results[c]["out"], dtype=np.float32)
    if _trace:
        _cache["last_result"] = res
    return out
